# revision 68
# baseline (speedup 1.0000x reference)
"""SimCLR contrastive loss (NT-Xent) on 8 Trainium2 NeuronCores.

Reference:
    z  = concat(z_i, z_j)                     # [N, D], N = 8192, D = 256
    zn = z / max(||z||_row, eps)
    sim = zn @ zn.T / TEMP                    # TEMP = 0.5
    lse = logsumexp(sim with -inf diagonal, axis=1)
    pos[r] = sim[r, (r + B) mod N]
    loss = sum(lse - pos) / N

Distribution: data-parallel over rows.  Core c owns rows [1024c, 1024c+1024);
the host ships each core the *row-rotated* embeddings so one SPMD program
serves all cores (own rows are always local columns [0, 1024), the positive
window for row-tile m is local columns [4096+128m, 4096+128m+128), and the
diagonal is folded out by subtracting the constant e^2 from each row sum).

Per-core pipeline (v3: three-engine consumers, SBUF inv broadcast):
  Host ships two bf16 layouts of the rotated z (pure layout work):
    ztp  [128, 2, 8192]: ztp[p, j, c] = z_loc[c, p + 128j]
    zrow [128, 16384]:   zrow[p, 256t + k] = z_loc[128t + p, k]
  1. Norms: group 0 split DVE (TT square 2x + 3D reduce) / ACT (Square with
     fused accum); groups 1,3 on gpsimd; group 2 on ACT interleaved between
     early exp tiles.  Quake-seed Newton rsqrt on DVE per *group* [128,16];
     DVE StreamTranspose (32x32 blocks) re-lays inv so a gpsimd strided DMA
     writes it contiguously to DRAM; a stride-0 (broadcast_to) HWDGE DMA
     then lands bc[g] = [128, 2048] bf16 in SBUF (no PSUM involved).
  2. znb = ztp * bc on DVE at 2x (bf16 SBUF everywhere) + CAST to fp8e4.
  3. Main loop (4 sweeps x 8 row tiles): [128, 2048] PSUM tiles from 4
     DoubleRow fp8 matmuls; consumed by ScalarE (exp + fused row-sum,
     scale=2 folds 1/TEMP), VectorE, or gpsimd (Schraudolph fast-exp:
     (x*A + B) -> int16 bit pattern == bf16 exp approximation, then row-sum).
  4. Batched finalization: one reduce/Ln pass over [128, 8] instead of
     per-row-tile small ops; out = lse - pos; host sums in fp64 / N.
"""

import os
import sys

import numpy as np

B = 4096
D = 256
N = 2 * B
NCORES = 8
RPC = N // NCORES  # rows per core

_CANDIDATE_PATHS = ("/opt/trn_rl_repo", "/root/.axon_site/_ro/trn_rl_repo")


def _ensure_import_path():
    try:
        import concourse.bass  # noqa: F401
        return
    except ImportError:
        pass
    for p in _CANDIDATE_PATHS:
        if os.path.isdir(p) and p not in sys.path:
            sys.path.insert(0, p)
    import concourse.bass  # noqa: F401


# Schraudolph fast-exp constants for exp(2*x) via bf16 bit pattern:
#   bits = round(x * 2*128*log2(e) + (127*128 - 128*log2(Eg)))
# where Eg = E_f[(1+f)/2^f] = 1.0406984 zeroes the mean sawtooth bias.
SCH_A = 369.32993046757464          # 2 * 128 * log2(e)
SCH_B = 16248.633                   # 16256 - 128*log2(1.0406984)
EXP2 = float(np.exp(2.0))           # exp(sim[i,i] * 2) subtracted per row


def build_program(n=N, d=D, rpc=RPC):
    _ensure_import_path()
    from contextlib import ExitStack

    import concourse.bacc as bacc
    import concourse.tile as tile
    from concourse import mybir

    f32 = mybir.dt.float32
    bf16 = mybir.dt.bfloat16
    fp8 = mybir.dt.float8e4
    i16 = mybir.dt.int16
    i32 = mybir.dt.int32
    FT = mybir.ActivationFunctionType
    OP = mybir.AluOpType
    DR = mybir.MatmulPerfMode.DoubleRow

    P = 128
    CH = 512                    # one fp32 PSUM bank
    GW = 2048                   # sweep/group width (4 banks)
    nsw = n // GW               # 4 sweeps
    mt = rpc // P               # 8 row tiles
    tpg = GW // P               # norm t-chunks per group (16)
    EG = GW // CH               # 512-chunks per sweep (4)

    # exp-tile consumers: ScalarE by default, these go to DVE.
    # gpsimd ALU is unusable here: it cannot read PSUM, and its software
    # loops contend SBUF so hard that concurrent DVE ops run ~40x slower.
    DVE_TILES = {(2, 2), (2, 6), (3, 3)}

    nc = bacc.Bacc("TRN2", target_bir_lowering=False, debug=False)
    ztp_d = nc.dram_tensor("ztp", [P, 2, n], bf16, kind="ExternalInput").ap()
    zrow_d = nc.dram_tensor("zrow", [P, (n // P) * d], bf16, kind="ExternalInput").ap()
    id_d = nc.dram_tensor("ident", [P, P], f32, kind="ExternalInput").ap()
    idb_d = nc.dram_tensor("identb", [P, P], bf16, kind="ExternalInput").ap()
    out_d = nc.dram_tensor("out", [P, mt], f32, kind="ExternalOutput").ap()
    invd = nc.dram_tensor("invd", [1, n], bf16).ap()   # scratch: 1/norm

    with tile.TileContext(nc) as tc, ExitStack() as ctx:
        big = ctx.enter_context(tc.tile_pool(name="big", bufs=1))
        small = ctx.enter_context(tc.tile_pool(name="small", bufs=2))
        i16p = ctx.enter_context(tc.tile_pool(name="i16p", bufs=2))
        stat = ctx.enter_context(tc.tile_pool(name="stat", bufs=1))
        mps = ctx.enter_context(tc.tile_pool(name="mps", bufs=2, space="PSUM"))

        zt = big.tile([P, 2, n], bf16, tag="zt")
        zrow = big.tile([P, (n // P) * d], bf16, tag="zrow")
        znb = big.tile([P, 2, n], fp8, tag="znb")
        znp = ctx.enter_context(tc.tile_pool(name="znp", bufs=2))
        zn_t = {}                                      # per-group normalize scratch
        jnk = big.tile([P, GW], bf16, tag="jnk")       # DVE exp-sum junk out
        sqb = big.tile([P, tpg * d], bf16, tag="sqb")  # DVE squares scratch
        ident_sb = stat.tile([P, P], f32, tag="ident")
        identb_sb = stat.tile([P, P], bf16, tag="identb")
        ones1 = stat.tile([1, P], bf16, tag="ones1")
        invT = stat.tile([tpg, P], bf16, tag="invT")
        invT1 = stat.tile([1, GW], bf16, tag="invT1")
        ssg = [stat.tile([P, tpg], f32, tag=f"ss{g}", name=f"ss{g}") for g in range(nsw)]
        invpr = [stat.tile([P, 2 * tpg], bf16, tag=f"inv{q}", name=f"inv{q}") for q in range(2)]
        stv = [stat.tile([P, 2 * tpg], bf16, tag=f"stv{q}", name=f"stv{q}") for q in range(2)]
        bc = [stat.tile([P, GW], bf16, tag=f"bc{g}", name=f"bc{g}") for g in range(nsw)]
        out_sb = stat.tile([P, mt], f32, tag="out_sb")
        partsA = stat.tile([P, mt, nsw], f32, tag="partsA")
        partsB = stat.tile([P, mt, nsw], f32, tag="partsB")
        poss = stat.tile([P, mt], f32, tag="poss")

        # ---- input streams (SP HWDGE queue, in priority order) ----
        # group-0 zrow in halves so norms start ASAP; then the rest in
        # few, large DMAs (fewer dispatches, fewer sem-pool aliases).
        hz = tpg // 2 * d
        nc.sync.dma_start(out=ident_sb, in_=id_d)
        nc.sync.dma_start(out=identb_sb, in_=idb_d)
        nc.sync.dma_start(out=zrow[:, 0:hz], in_=zrow_d[:, 0:hz])
        nc.sync.dma_start(out=zrow[:, hz : 2 * hz], in_=zrow_d[:, hz : 2 * hz])
        nc.sync.dma_start(out=zt[:, :, 0:GW], in_=ztp_d[:, :, 0:GW])
        zr = slice(tpg * d, 2 * tpg * d)
        nc.sync.dma_start(out=zrow[:, zr], in_=zrow_d[:, zr])
        zr = slice(2 * tpg * d, 4 * tpg * d)
        nc.sync.dma_start(out=zrow[:, zr], in_=zrow_d[:, zr])
        nc.sync.dma_start(out=zt[:, :, GW : 2 * GW], in_=ztp_d[:, :, GW : 2 * GW])
        nc.sync.dma_start(out=zt[:, :, 2 * GW :], in_=ztp_d[:, :, 2 * GW :])

        nc.vector.memset(ones1, 1.0)
        nc.vector.memset(partsA, 0.0)
        nc.vector.memset(partsB, 0.0)
        nc.vector.memset(invpr[0], 0.0)
        nc.vector.memset(invpr[1], 0.0)

        # ---- norms ----
        def gate(src_ap):
            """Tiny DVE op reading src_ap and writing the norms scratch: a
            real dependency that stops the Tile scheduler from front-running
            later norm groups ahead of the critical normalize chains."""
            nc.vector.tensor_copy(out=sqb[:, 0:2], in_=src_ap)

        def norms_dve(g, t0, t1):
            """ss for t-chunks [t0, t1) of group g: TT square (2x bf16)
            + 3D reduce over the half-open chunk range."""
            zc = slice((tpg * g + t0) * d, (tpg * g + t1) * d)
            sc = slice(t0 * d, t1 * d)
            nc.vector.tensor_mul(sqb[:, sc], zrow[:, zc], zrow[:, zc])
            nc.vector.tensor_reduce(
                out=ssg[g][:, t0:t1],
                in_=sqb[:, sc].rearrange("p (t d) -> p t d", d=d),
                axis=mybir.AxisListType.X,
                op=OP.add,
            )

        def newton(g, st=True):
            """inv = 1/sqrt(ss) for group g: Quake seed + 1 Newton step
            (seed err ~3.4% -> ~0.17% after one step, below the bf16
            quantization of bc), written into invpr[g//2] as bf16."""
            q, h = g // 2, g % 2
            sg = ssg[g]
            ii = small.tile([P, tpg], i32, tag="ii")
            nc.vector.tensor_scalar(
                out=ii, in0=sg.bitcast(i32), scalar1=1, scalar2=None,
                op0=OP.arith_shift_right,
            )
            nc.vector.tensor_scalar(
                out=ii, in0=ii, scalar1=-1, scalar2=None, op0=OP.bitwise_xor
            )
            nc.vector.tensor_scalar(
                out=ii, in0=ii, scalar1=0x5F3759DF + 1, scalar2=None, op0=OP.add
            )
            y = ii.bitcast(f32)
            t_ = small.tile([P, tpg], f32, tag="t_")
            nc.vector.tensor_mul(t_, y, y)
            nc.vector.tensor_mul(t_, t_, sg)
            nc.vector.tensor_scalar(
                out=t_, in0=t_, scalar1=-0.5, scalar2=1.5,
                op0=OP.mult, op1=OP.add,
            )
            nc.vector.tensor_mul(y, y, t_)
            nc.vector.tensor_copy(out=invpr[q][:, tpg * h : tpg * (h + 1)], in_=y)
            if st:
                # 32x32-block stream transpose: stv[32k+i,j] = invpr[32k+j,i]
                # so stv partition 32k+i holds inv of rows 128i+32k+j.
                nc.vector.transpose(stv[q], invpr[q])

        pe_bc_tiles = {}

        def bc_via_pe(g):
            """Broadcast group g's inv through the PE: transpose the compact
            invpr half -> invT[t, p] = inv(row 128t+p) (flattened this IS
            row-ordered), then 16 ones-matmuls broadcast it to a [128, 2048]
            PSUM tile.  No DRAM scatter (6us HBM RMW) or stride-0 HBM
            broadcast (6us).  The two PSUM tiles are allocated once and
            reused for later groups (WAR deps order the reuse)."""
            q, h = g // 2, g % 2
            if not pe_bc_tiles:
                pe_bc_tiles["tr"] = mps.tile([P, GW], f32, tag="ps", name="tr")
                pe_bc_tiles["bcps"] = mps.tile([P, GW], f32, tag="ps", name="bcps")
            tr, bcps = pe_bc_tiles["tr"], pe_bc_tiles["bcps"]
            trv = tr[:, 0 : P // 2].bitcast(bf16)
            nc.tensor.transpose(
                trv[0:tpg, :], invpr[q][:, tpg * h : tpg * (h + 1)], identb_sb
            )
            nc.vector.tensor_copy(out=invT, in_=trv[0:tpg, 0:P])
            # collapse [16, 128] onto partition 0 (matmul moving operands
            # must start at partition 0/32/64): 16 SBUF-to-SBUF descriptors
            nc.gpsimd.dma_start(
                out=invT1[0:1, :].rearrange("o (t p) -> o t p", t=tpg),
                in_=invT,
            )
            for c in range(EG):
                nc.tensor.matmul(
                    bcps[:, CH * c : CH * (c + 1)],
                    ones1,
                    invT1[0:1, CH * c : CH * (c + 1)],
                    start=True,
                    stop=True,
                )
            return bcps

        def invd_write(g):
            """Contiguous-ish DRAM write (64B runs) of the whole pair holding
            group g; the other group's half is garbage until its newton runs,
            but bc_load only ever reads halves whose newton is done."""
            q = g // 2
            # stv[p, j] = inv(row 128*(p%32) + 32*(p//32) + j) within pair q
            dst = invd[0:1, 2 * GW * q : 2 * GW * (q + 1)].rearrange(
                "o (i k j) -> o k i j", i=2 * tpg, k=4
            )
            nc.gpsimd.dma_start(out=dst, in_=stv[q])

        def bc_load(g, first=None):
            """Stride-0 broadcast DMA: invd[1, G] -> bc[g] [128, G]."""
            G0 = GW * g
            if first is None:
                nc.gpsimd.dma_start(
                    out=bc[g], in_=invd[0:1, G0 : G0 + GW].broadcast_to((P, GW))
                )
            else:
                nc.gpsimd.dma_start(
                    out=bc[g][:, 0:first],
                    in_=invd[0:1, G0 : G0 + first].broadcast_to((P, first)),
                )
                nc.gpsimd.dma_start(
                    out=bc[g][:, first:GW],
                    in_=invd[0:1, G0 + first : G0 + GW].broadcast_to((P, GW - first)),
                )

        def normalize_chunk(g, c0, c1, cast="dve"):
            """znb[:, :, c0:c1] = zt * bc: one TT per j writing fp8 directly
            (no bf16 scratch / separate cast -- shortens the chain by the
            ~5us cast-DMA latency).  zn_t still tracks a tiny marker tile so
            the gates have something to read."""
            if g not in zn_t:
                zn_t[g] = znp.tile([P, 2, GW], bf16, tag="zn16", name=f"zn{g}")
            G = slice(GW * g + c0, GW * g + c1)
            for j in range(2):
                nc.vector.tensor_mul(znb[:, j, G], zt[:, j, G], bc[g][:, c0:c1])
            nc.vector.tensor_copy(out=zn_t[g][:, 0, 0:2], in_=znb[:, 0, GW * g : GW * g + 2])

        def normalize_chunk_ps(g, c0, c1, bcps, cast="dve"):
            """Like normalize_chunk but reading inv from a PSUM broadcast
            tile (groups whose bc went through the PE)."""
            if g not in zn_t:
                zn_t[g] = znp.tile([P, 2, GW], bf16, tag="zn16", name=f"zn{g}")
            zn16 = zn_t[g]
            G = slice(GW * g + c0, GW * g + c1)
            zc = slice(c0, c1)
            for j in range(2):
                nc.vector.tensor_mul(zn16[:, j, zc], zt[:, j, G], bcps[:, c0:c1])
            if cast == "dve":
                nc.vector.tensor_copy(out=znb[:, :, G], in_=zn16[:, :, zc])
            else:
                nc.gpsimd.dma_start(out=znb[:, :, G], in_=zn16[:, :, zc])

        def warmup(src, k, nmm):
            """Dummy matmuls on already-loaded data keep the PE HAM warm
            (any >3.4us idle gap drops the PE clock 2.4 -> 1.2 GHz)."""
            wps = mps.tile([P, GW], f32, tag="ps", name=f"wu{k}")
            for i in range(nmm):
                nc.tensor.matmul(
                    wps[:, 0:CH], src[:, 0:P], src[:, 0:CH],
                    start=True, stop=True,
                )

        # ---- prologue ----
        norms_dve(0, 0, tpg // 2)
        warmup(zrow, 0, 2)
        norms_dve(0, tpg // 2, tpg)
        newton(0)
        invd_write(0)
        bc_load(0, first=CH)
        warmup(zrow[:, tpg * d :], 1, 2)
        gate(stv[0][0:128, 0:2])   # norms g1 only after newton(0)+ST done
        norms_dve(1, 0, tpg)       # fills the bc0 DMA-roundtrip gap
        normalize_chunk(0, 0, CH)
        normalize_chunk(0, CH, 2 * CH)
        warmup(zt[:, 0, :], 2, 2)
        normalize_chunk(0, 2 * CH, 3 * CH)
        normalize_chunk(0, 3 * CH, GW)

        # ---- main loop ----
        def main_tile(s, m):
            ps = mps.tile([P, GW], f32, tag="ps", name=f"ps_{s}_{m}")
            for c in range(EG):
                cols = slice(GW * s + CH * c, GW * s + CH * (c + 1))
                nc.tensor.matmul(
                    ps[:, CH * c : CH * (c + 1)],
                    znb[:, :, P * m : P * (m + 1)],
                    znb[:, :, cols],
                    start=True,
                    stop=True,
                    perf_mode=DR,
                )
            w0 = n // 2 + P * m
            if w0 // GW == s:  # positive-pair window lives in this sweep
                off = w0 % GW
                junk = small.tile([P, P], f32, tag="pjunk")
                nc.vector.scalar_tensor_tensor(
                    out=junk,
                    in0=ps[:, off : off + P],
                    scalar=2.0,
                    in1=ident_sb,
                    op0=OP.mult,
                    op1=OP.mult,
                    accum_out=poss[:, m : m + 1],
                )
            if (s, m) in DVE_TILES:
                ib = i16p.tile([P, GW], i16, tag="ib", name=f"ib_{s}_{m}")
                nc.vector.tensor_scalar(
                    out=ib, in0=ps, scalar1=SCH_A, scalar2=SCH_B,
                    op0=OP.mult, op1=OP.add,
                )
                nc.vector.tensor_scalar(
                    out=jnk, in0=ib.bitcast(bf16), scalar1=1.0, scalar2=None,
                    op0=OP.mult, op1=OP.add,
                    accum_out=partsB[:, m, s : s + 1],
                )
            else:
                nc.scalar.activation(
                    out=ps,
                    in_=ps,
                    func=FT.Exp,
                    scale=2.0,
                    accum_out=partsA[:, m, s : s + 1],
                )

        tseq = [(s, m) for s in range(nsw) for m in range(mt)]
        for k, (s, m) in enumerate(tseq):
            main_tile(s, m)
            if k == 0:
                newton(1)
                invd_write(1)
                bc_load(1)
            elif k == 1:
                normalize_chunk(1, 0, GW, cast="gp")
            elif k == 2:
                gate(zn_t[1][0:128, 0, 0:2])
                norms_dve(2, 0, tpg)
            elif k == 4:
                newton(2)
                invd_write(2)
                bc_load(2)
            elif k == 5:
                normalize_chunk(2, 0, GW, cast="gp")
            elif k == 6:
                gate(zn_t[2][0:128, 0, 0:2])
                norms_dve(3, 0, tpg)
            elif k == 8:
                newton(3)
                invd_write(3)
                bc_load(3)
            elif k == 9:
                normalize_chunk(3, 0, GW, cast="gp")

        # ---- batched finalization ----
        SA = stat.tile([P, mt, 1], f32, tag="SA")
        SB = stat.tile([P, mt, 1], f32, tag="SB")
        nc.vector.tensor_reduce(
            out=SA, in_=partsA, axis=mybir.AxisListType.X, op=OP.add
        )
        nc.vector.tensor_reduce(
            out=SB, in_=partsB, axis=mybir.AxisListType.X, op=OP.add
        )
        nc.vector.tensor_add(SA, SA, SB)
        nc.vector.tensor_scalar_add(SA, SA, -EXP2)
        lse = stat.tile([P, mt], f32, tag="lse")
        nc.scalar.activation(out=lse, in_=SA.rearrange("p m o -> p (m o)"), func=FT.Ln)
        nc.vector.tensor_tensor(
            out=out_sb, in0=lse, in1=poss, op=OP.subtract,
        )
        nc.sync.dma_start(out=out_d, in_=out_sb)

    nc.compile()
    return nc


def make_in_maps(z_i, z_j, n=N, d=D, rpc=RPC, ncores=NCORES):
    """Host-side sharding: two rotated bf16 layouts per core (layout only)."""
    import ml_dtypes

    P = 128
    z = np.concatenate(
        [np.asarray(z_i, dtype=np.float32), np.asarray(z_j, dtype=np.float32)],
        axis=0,
    )
    ident = np.eye(P, dtype=np.float32)
    in_maps = []
    for c in range(ncores):
        z_loc = np.roll(z, -rpc * c, axis=0)              # [N, D]
        zT = z_loc.T                                      # [D, N]
        ztp = np.ascontiguousarray(
            zT.reshape(2, P, n).transpose(1, 0, 2)
        ).astype(ml_dtypes.bfloat16)                      # [128, 2, N]
        zrow = np.ascontiguousarray(
            z_loc.reshape(n // P, P, d).transpose(1, 0, 2).reshape(P, -1)
        ).astype(ml_dtypes.bfloat16)                      # [128, (N/128)*D]
        in_maps.append({"ztp": ztp, "zrow": zrow, "ident": ident,
                        "identb": ident.astype(ml_dtypes.bfloat16)})
    return in_maps


def gather_loss(results, n=N):
    total = 0.0
    for r in results:
        total += np.asarray(r["out"], dtype=np.float64).sum()
    return np.float32(total / n)


_PROGRAM_CACHE = {}


def kernel(z_i, z_j):
    _ensure_import_path()
    from concourse.bass_utils import run_bass_kernel_spmd

    key = (N, D, RPC)
    if key not in _PROGRAM_CACHE:
        _PROGRAM_CACHE[key] = build_program()
    nc = _PROGRAM_CACHE[key]
    in_maps = make_in_maps(z_i, z_j)
    results = run_bass_kernel_spmd(nc, in_maps, list(range(NCORES))).results
    return gather_loss(results)


if __name__ == "__main__":
    rng = np.random.default_rng(0)
    z_i = rng.standard_normal((B, D), dtype=np.float32)
    z_j = rng.standard_normal((B, D), dtype=np.float32)
    loss = kernel(z_i, z_j)
    print("loss:", loss)
